# revision 25
# baseline (speedup 1.0000x reference)
"""DGCNN (dynamic-graph edge conv) Trainium2 Bass kernel — 8-core version.

Two cores per sample (8 cores, B=4). Each core processes half the points
(2048 rows of the kNN/top-k/edge-conv work); point columns are host-rolled
per core so the SPMD program is identical on all cores ("local order").
Per edge-conv layer:
  - keys key[i,j] = <f_i, f_j> - |f_j|^2/2 on the PE (float32r fast path),
    one 128-row block at a time over all 4096 columns.
  - exact top-20 per row on the DVE: 3 rounds of max8 / max_index /
    match_replace.  The DVE does *only* this; all other elementwise work
    lives on gpsimd (Pool) / ACT so top-k sets the pipeline rate.
  - linearized edge conv h_ik = a_i + b_j(i,k); max_k and the GroupNorm
    stats are computed from the gathered b rows alone:
      max_k h = a + max_k b ;  sum_k h = K a + S_b ;
      sum_k h^2 = a (K a + 2 S_b) + S_bq.
    b rows staged to DRAM, fetched with SWDGE dma_gather.
  - halves exchanged with a pair AllGather; per-channel GN stats ride along
    as 2 extra f32 columns per feature row (elem size 2112); the unpack is a
    dma_gather whose index tile is a per-core host input (handles the
    rank-dependent chunk order).
  - GroupNorm affine (sign-folded so max commutes) + leaky applied on both
    halves after the exchange (ACT affine + one gpsimd max(0.2x, x) pass).
Aggregation conv + pooling + the conditioned MLP head run redundantly on
both cores of a pair from the full (local-order) features; the host reads
the even core's output.
"""

import os
from contextlib import ExitStack

import numpy as np

import concourse.bacc as bacc
import concourse.bass as bass
import concourse.mybir as mybir
import concourse.tile as tile

F32 = mybir.dt.float32
F32R = mybir.dt.float32r
I16 = mybir.dt.int16
U16 = mybir.dt.uint16

P = 128
K = 20
G = 8
EPS = 1e-5
NEG = -3.0e38
HN = 2048          # local half of the points
FW = 2112          # feature row width in the collective: 2048 feat + 2 stats + pad
RG = [[0, 1], [2, 3], [4, 5], [6, 7]]

# (Cin, Cout) per edge conv layer.
LAYERS = [(8, 64), (64, 64), (64, 128)]


def _sf(x):
    return np.ascontiguousarray(x, dtype=np.float32)


def host_prep(inputs, N):
    """Derive all constant tensors shipped to every core (rank-independent)."""
    c = {}
    c["eye"] = _sf(np.eye(P))
    c["nh128"] = _sf(np.full((P, 1), -0.5))
    c["one11"] = _sf(np.ones((1, 1)))
    # S_q: out = S_q.T @ T places T[16q+p, k] at out[16G+p, k] for all G
    for q in range(8):
        m = np.zeros((P, P), np.float32)
        for Gg in range(8):
            for p in range(16):
                m[16 * q + p, 16 * Gg + p] = 1.0
        c[f"Sq{q}"] = _sf(m)

    Ws = [(inputs["W1"], inputs["g1"], inputs["b1"]),
          (inputs["W2"], inputs["g2"], inputs["b2"]),
          (inputs["W3"], inputs["g3"], inputs["b3"])]
    for l, ((W, g, b), (Cin, Cout)) in enumerate(zip(Ws, LAYERS)):
        Wc, Wn = W[:, :Cin], W[:, Cin:]
        sign = np.where(g >= 0, 1.0, -1.0).astype(np.float32)
        A = ((Wc - Wn) * sign[:, None]).T  # (Cin, Cout), sign folded
        Bm = (Wn * sign[:, None]).T
        Kr = 32 if l == 0 else Cin  # ones/halfsq row (32-aligned for DVE/ACT)
        Aw = np.zeros((Kr + 1, Cout), np.float32)
        Bw = np.zeros((Kr + 1, Cout), np.float32)
        Aw[:Cin, :] = A
        Bw[:Cin, :] = Bm
        c[f"Aw{l}"] = _sf(Aw)
        c[f"Bw{l}"] = _sf(Bw)
        # group indicator matrices
        cpg = Cout // G
        gh = np.zeros((Cout, G), np.float32)
        for gg in range(G):
            gh[gg * cpg:(gg + 1) * cpg, gg] = 1.0
        c[f"GHs{l}"] = _sf(gh * sign[:, None])
        c[f"GHu{l}"] = _sf(gh)
        c[f"GT{l}"] = _sf(gh.T)
        c[f"absg{l}"] = _sf(np.abs(g)[:, None])
        c[f"g{l}"] = _sf(g[:, None])
        c[f"beta{l}"] = _sf(b[:, None])

    # aggregation conv Wa (256, 256) split into 2 output halves x 3 in chunks
    Wa = inputs["Wa"]
    chunks = [(0, 64), (64, 128), (128, 256)]
    for h in range(2):
        for ci, (s, e) in enumerate(chunks):
            c[f"WaT{h}{ci}"] = _sf(Wa[h * 128:(h + 1) * 128, s:e].T)
    ga, ba = inputs["ga"], inputs["ba"]
    for h in range(2):
        gh = np.zeros((128, G), np.float32)
        for gg in range(4):
            gh[gg * 32:(gg + 1) * 32, 4 * h + gg] = 1.0
        c[f"GHa{h}"] = _sf(gh)
        gt = np.zeros((G, 128), np.float32)
        for gg in range(4):
            gt[4 * h + gg, gg * 32:(gg + 1) * 32] = 1.0
        c[f"GTa{h}"] = _sf(gt)
        c[f"ga{h}"] = _sf(ga[h * 128:(h + 1) * 128][:, None])
        c[f"betaa{h}"] = _sf(ba[h * 128:(h + 1) * 128][:, None])

    def gn_consts(prefix, C, gamma, beta):
        cpg = C // G
        gh = np.zeros((C, G), np.float32)
        for gg in range(G):
            gh[gg * cpg:(gg + 1) * cpg, gg] = 1.0
        c[f"GH{prefix}"] = _sf(gh)
        c[f"GT{prefix}"] = _sf(gh.T)
        c[f"g{prefix}"] = _sf(gamma[:, None])
        c[f"beta{prefix}"] = _sf(beta[:, None])

    gn_consts("c1", 64, inputs["gc1"], inputs["bc1"])
    gn_consts("c2", 64, inputs["gc2"], inputs["bc2"])
    gn_consts("z2", 128, inputs["gs2"], inputs["bs2"])
    gn_consts("z3", 64, inputs["gs3"], inputs["bs3"])
    # z1 (256 ch) as two halves of 128, groups of 32
    gs1, bs1 = inputs["gs1"], inputs["bs1"]
    for h in range(2):
        gh = np.zeros((128, G), np.float32)
        gt = np.zeros((G, 128), np.float32)
        for gg in range(4):
            gh[gg * 32:(gg + 1) * 32, 4 * h + gg] = 1.0
            gt[4 * h + gg, gg * 32:(gg + 1) * 32] = 1.0
        c[f"GHz1{h}"] = _sf(gh)
        c[f"GTz1{h}"] = _sf(gt)
        c[f"gz1{h}"] = _sf(gs1[h * 128:(h + 1) * 128][:, None])
        c[f"betaz1{h}"] = _sf(bs1[h * 128:(h + 1) * 128][:, None])

    c["Wc1T"] = _sf(inputs["Wc1"].T)   # (2, 64)
    c["Wc2T"] = _sf(inputs["Wc2"].T)   # (64, 64)
    # Ws1 (256, 576): z order [max0(128) max1(128) mean0(128) mean1(128) c2(64)]
    Ws1 = inputs["Ws1"]
    zchunks = [(0, 128), (128, 256), (256, 384), (384, 512), (512, 576)]
    for h in range(2):
        for ci, (s, e) in enumerate(zchunks):
            c[f"Ws1T{h}{ci}"] = _sf(Ws1[h * 128:(h + 1) * 128, s:e].T)
    Ws2 = inputs["Ws2"]
    for h in range(2):
        c[f"Ws2T{h}"] = _sf(Ws2[:, h * 128:(h + 1) * 128].T)
    c["Ws3T"] = _sf(inputs["Ws3"].T)   # (128, 64)
    c["Ws4T"] = _sf(inputs["Ws4"].T)   # (64, 2)
    c["bs4"] = _sf(inputs["bs4"][:, None])
    c["neghalf"] = _sf(np.full((64, 1), -0.5))
    return c


def host_gidx(rank):
    """Per-core unpack-gather index tiles, (128, 48) f32.

    Layer l: cols [16l:16l+8] = self rows, [16l+8:16l+16] = peer rows of the
    allgathered (2C, FW) table; value -1 marks skipped trailing slots.
    """
    m = np.full((128, 48), -1.0, np.float32)
    for l, (_, Cout) in enumerate(LAYERS):
        for i in range(Cout):
            r, s = i % 16, i // 16
            for g in range(8):
                m[16 * g + r, 16 * l + s] = rank * Cout + i
                m[16 * g + r, 16 * l + 8 + s] = (1 - rank) * Cout + i
    return _sf(m)


def const_shapes(N):
    """Shapes of every constant input (dict name -> shape)."""
    fake = {
        "W1": np.zeros((64, 16)), "g1": np.zeros(64), "b1": np.zeros(64),
        "W2": np.zeros((64, 128)), "g2": np.zeros(64), "b2": np.zeros(64),
        "W3": np.zeros((128, 128)), "g3": np.zeros(128), "b3": np.zeros(128),
        "Wa": np.zeros((256, 256)), "ga": np.zeros(256), "ba": np.zeros(256),
        "Wc1": np.zeros((64, 2)), "gc1": np.zeros(64), "bc1": np.zeros(64),
        "Wc2": np.zeros((64, 64)), "gc2": np.zeros(64), "bc2": np.zeros(64),
        "Ws1": np.zeros((256, 576)), "gs1": np.zeros(256), "bs1": np.zeros(256),
        "Ws2": np.zeros((128, 256)), "gs2": np.zeros(128), "bs2": np.zeros(128),
        "Ws3": np.zeros((64, 128)), "gs3": np.zeros(64), "bs3": np.zeros(64),
        "Ws4": np.zeros((2, 64)), "bs4": np.zeros(2),
    }
    return {k: v.shape for k, v in host_prep(fake, N).items()}


def build_nc(N, num_devices=8, ablate=frozenset(), repeat=1):
    """Build the single-core SPMD program (half a sample per core)."""
    nc = bacc.Bacc("TRN2", target_bir_lowering=False, debug=False,
                   num_devices=num_devices, num_swdge_queues=4)
    nblk = N // P          # 32: b-table blocks (full point set)
    nloc = HN // P         # 16: local row blocks

    dram = {}
    dram["x"] = nc.dram_tensor("x", [8, N], F32, kind="ExternalInput")
    dram["cond"] = nc.dram_tensor("cond", [2, 1], F32, kind="ExternalInput")
    dram["gidx"] = nc.dram_tensor("gidx", [128, 48], F32, kind="ExternalInput")
    for name, shape in const_shapes(N).items():
        dram[name] = nc.dram_tensor(name, list(shape), F32, kind="ExternalInput")
    out_d = nc.dram_tensor("out", [2, 1], F32, kind="ExternalOutput")
    for l, (_, Cout) in enumerate(LAYERS):
        dram[f"_bt{l}"] = nc.dram_tensor(f"_bt{l}", [N + 1, 128], F32)
        dram[f"_ci{l}"] = nc.dram_tensor(f"_ci{l}", [Cout, FW], F32)
        dram[f"_co{l}"] = nc.dram_tensor(f"_co{l}", [2 * Cout, FW], F32)

    with tile.TileContext(nc) as tc:
        with ExitStack() as ctx:
            emit(ctx, tc, nc, dram, out_d, N, nblk, nloc, ablate, repeat)
    nc.compile()
    return nc


def emit(ctx, tc, nc, dram, out_d, N, nblk, nloc, ablate=frozenset(), repeat=1):
    cpool = ctx.enter_context(tc.tile_pool(name="consts", bufs=1))
    fpool = ctx.enter_context(tc.tile_pool(name="feat", bufs=1))
    wpool = ctx.enter_context(tc.tile_pool(name="work", bufs=2))
    w3pool = ctx.enter_context(tc.tile_pool(name="work3", bufs=3))
    spool = ctx.enter_context(tc.tile_pool(name="small", bufs=2))
    pk = ctx.enter_context(tc.tile_pool(name="pkt", bufs=1, space="PSUM"))
    pidxp = ctx.enter_context(tc.tile_pool(name="pidx", bufs=2, space="PSUM"))
    psp = ctx.enter_context(tc.tile_pool(name="psmall", bufs=2, space="PSUM"))

    V = nc.vector
    S = nc.scalar
    T = nc.tensor
    Y = nc.sync
    GP = nc.gpsimd

    def r32(ap):
        return ap  # f32r needs rounded producers; keep plain f32 for now

    # ---- load constants ----
    cs = {}
    for name, d in dram.items():
        if name in ("x", "cond", "gidx") or name.startswith("_"):
            continue
        t = cpool.tile(list(d.shape), F32, tag=f"c_{name}")
        Y.dma_start(t[:, :], d[:, :])
        cs[name] = t
    cond_sb = cpool.tile([2, 1], F32, tag="c_cond")
    Y.dma_start(cond_sb[:, :], dram["cond"][:, :])
    gidxf = cpool.tile([128, 48], F32, tag="c_gidxf")
    Y.dma_start(gidxf[:, :], dram["gidx"][:, :])
    gidx = cpool.tile([128, 48], I16, tag="c_gidx")
    GP.tensor_copy(gidx[:, :], gidxf[:, :])

    # ---- persistent feature tiles ----
    # f0: local-order input x (host-rolled), rows 0:8 = x, 8:32 zero, 32 ones.
    f0 = fpool.tile([33, N], F32, tag="f0")
    V.memset(f0[0:33, :], 0.0)
    Y.dma_start(f0[0:8, :], dram["x"][:, :])
    V.memset(f0[32:33, :], 1.0)
    # f1..f3: [0:FW] = half A {feat 2048 | stats 2 | pad}, [FW:2FW] = half B.
    f1 = fpool.tile([128, 2 * FW], F32, tag="f1")
    f2 = fpool.tile([128, 2 * FW], F32, tag="f2")
    f3 = fpool.tile([128, 2 * FW], F32, tag="f3")
    V.memset(f1[64:65, 0:HN], 1.0)   # ones row for next layer's lhsT
    V.memset(f2[64:65, 0:HN], 1.0)
    ftiles = [f0, f1, f2, f3]

    def fcols(t, rows, j0, w):
        """AP for rows [0, rows) x feature cols [j0, j0+w) in local order."""
        if t is f0:
            return t[0:rows, j0:j0 + w]
        if j0 < HN:
            assert j0 + w <= HN
            return t[0:rows, j0:j0 + w]
        return t[0:rows, FW + (j0 - HN):FW + (j0 - HN) + w]

    def psum_to_sbuf(dst_ap, src_psum, accum=None):
        S.activation(dst_ap, src_psum, mybir.ActivationFunctionType.Identity,
                     accum_out=accum)

    Hf = N // 2
    Qf = N // 4  # psum round width (2 banks)

    for _rep in range(repeat):
      # ================= edge conv layers =================
      for l, (Cin, Cout) in enumerate(LAYERS):
          fl = ftiles[l]
          fn = ftiles[l + 1]
          Kr = 32 if l == 0 else Cin
          Kc = Kr + 1

          # frhs: rows 0:Cin features (full N), row Kr = -0.5|f|^2
          frhs = fpool.tile([66, N], F32, tag="frhs")
          if l == 0:
              V.memset(frhs[0:33, :], 0.0)
              Y.dma_start(frhs[0:8, :], dram["x"][:, :])
          else:
              S.activation(frhs[0:Cin, 0:HN], fl[0:Cin, 0:HN],
                           mybir.ActivationFunctionType.Identity)
              S.activation(frhs[0:Cin, HN:N], fl[0:Cin, FW:FW + HN],
                           mybir.ActivationFunctionType.Identity)

          # -0.5*|f_j|^2 row
          fsq = wpool.tile([Cin, N], F32, tag="keysb")
          S.activation(fsq[:, 0:N], frhs[0:Cin, 0:N],
                       mybir.ActivationFunctionType.Square)
          for rr in range(4):
              kt = pk.tile([P, Qf], F32, tag="kt")
              for j0 in range(0, Qf, 512):
                  j1 = min(j0 + 512, Qf)
                  T.matmul(kt[0:1, j0:j1],
                           lhsT=cs["neghalf"][0:Cin, :],
                           rhs=fsq[:, rr * Qf + j0: rr * Qf + j1])
              psum_to_sbuf(frhs[Kr:Kr + 1, rr * Qf:(rr + 1) * Qf], kt[0:1, :])

          # b table rows (f32) -> DRAM; sentinel row N = 0
          bt_d = dram[f"_bt{l}"]
          zrow = spool.tile([1, 128], F32, tag="zrow")
          V.memset(zrow[:, :], 0.0)
          Y.dma_start(bt_d[N:N + 1, 0:Cout], zrow[:, 0:Cout])
          for bg in range(4):
              pb = pk.tile([P, Qf], F32, tag="kt")
              for j in range(8):
                  blk = 8 * bg + j
                  T.matmul(pb[:, Cout * j:Cout * (j + 1)],
                           lhsT=r32(fcols(fl, Cin, blk * P, P)),
                           rhs=r32(cs[f"Bw{l}"][0:Cin, 0:Cout]),
                           skip_group_check=True)
              btile = wpool.tile([P, 8, 128], F32, tag="btile")
              psum_to_sbuf(btile[:, :, 0:Cout],
                           pb[:, 0:8 * Cout].rearrange("p (j c) -> p j c", j=8))
              dst = bt_d[8 * P * bg:8 * P * (bg + 1), 0:Cout]
              Y.dma_start(dst.rearrange("(j p) c -> p j c", p=P),
                          btile[:, :, 0:Cout])

          # per-group sums accumulate in psum across all local blocks
          acc_s = pk.tile([1, 128], F32, tag="acc_s")
          acc_q = pk.tile([1, 128], F32, tag="acc_q")

          # ---- per local row-block, software-pipelined ----
          # Stage A: keys matmuls + PSUM->SBUF copies + a-block + DVE top-k.
          # Stage B: index pattern, SWDGE gather, k-reductions, stats.
          # B(i) is emitted two blocks behind A so the DVE never waits on a
          # gather (engines execute in emission order).
          def stageA(blk):
              lhsT = fcols(fl, Kc, blk * P, P)
              lhsTc = fcols(fl, Cin, blk * P, P)
              keysb = wpool.tile([P, N], F32, tag="keysb")
              for rr in range(4):
                  kt = pk.tile([P, Qf], F32, tag="kt")
                  for j0 in range(0, Qf, 512):
                      j1 = min(j0 + 512, Qf)
                      slq = slice(rr * Qf + j0, rr * Qf + j1)
                      T.matmul(kt[:, j0:j1], lhsT=r32(lhsT),
                               rhs=r32(frhs[0:Kc, slq]))
                  psum_to_sbuf(keysb[:, rr * Qf:(rr + 1) * Qf], kt[:, :])

              # a column block for these 128 points: aT[pt, c]
              pa = psp.tile([P, 128], F32, tag="ps")
              T.matmul(pa[:, 0:Cout], lhsT=r32(lhsTc),
                       rhs=r32(cs[f"Aw{l}"][0:Cin, :]))
              aT = w3pool.tile([P, 128], F32, tag="aT")
              psum_to_sbuf(aT[:, 0:Cout], pa[:, 0:Cout])

              # top-24 (indices of top-20 used) — DVE only
              Tw = w3pool.tile([P, 32], U16, tag="Tw")
              if "topk" in ablate:
                  V.memset(Tw[:, :], 0)
              else:
                  mx8 = wpool.tile([P, 8], F32, tag="mx8")
                  for r in range(3):
                      V.max(out=mx8[:, :], in_=keysb[:, :])
                      V.max_index(Tw[:, r * 8:(r + 1) * 8], mx8[:, :], keysb[:, :])
                      if r < 2:
                          V.match_replace(out=keysb[:, :], in_to_replace=mx8[:, :],
                                          in_values=keysb[:, :], imm_value=NEG)
              return {"aT": aT, "Tw": Tw}

          def stageB1(blk, st):
              Tw = st["Tw"]
              # T32f: f32 copy of indices, cols 20..31 = -1 (skipped by gather)
              T32 = wpool.tile([P, 32], F32, tag="T32")
              GP.tensor_copy(T32[:, 0:20], Tw[:, 0:20])
              GP.memset(T32[:, 20:32], -1.0)

              # idx pattern: iw[p, 8k+q] = T[16q+p, k]  (j = 16*(8k+q)+p = k*128+pt)
              pidx = pidxp.tile([P, 256], F32, tag="pidx")
              for q in range(8):
                  T.matmul(pidx[:, 32 * q:32 * (q + 1)],
                           lhsT=cs[f"Sq{q}"][:, :], rhs=T32[:, :])
              iw = wpool.tile([P, 256], I16, tag="iw")
              src = pidx[:, :].rearrange("p (q k) -> p k q", q=8)
              S.activation(iw[:, :].rearrange("p (k q) -> p k q", q=8), src,
                           mybir.ActivationFunctionType.Identity)

              # gather rows: gdst[pt, k, :] = bt[idx[pt, k], :]
              gdst = wpool.tile([P, 32, Cout], F32, tag="gdst")
              if "gather" in ablate:
                  V.memset(gdst[:, 0:1, 0:1], 0.0)
              else:
                  for rg, nvalid in ((0, P * 8), (1, P * 8), (2, P * 4)):
                      nc.gpsimd.dma_gather(
                          gdst[:, 8 * rg:8 * (rg + 1), :], bt_d[:, 0:Cout],
                          iw[:, 64 * rg:64 * (rg + 1)],
                          num_idxs=P * 8, num_idxs_reg=nvalid,
                          elem_size=Cout, elem_step=128, transpose=False,
                          single_packet=False, queue_num=(3 * blk + rg) % 4)
              st["gdst"] = gdst

          def stageB2(blk, st):
              aT, gdst = st["aT"], st["gdst"]
              # k-reduction tree on gpsimd (X-axis reduce is DVE-only).
              # Scratch = gdst k-slots 20:30 (gather pads, never written);
              # each tree's result (slot 20) is consumed before the next
              # tree reuses the scratch.  Pool engine wants 2-D contiguous
              # APs, and all k-slot ranges are contiguous in k-major layout.
              gf = gdst[:, :, :].rearrange("p k c -> p (k c)")
              C_ = Cout

              def sl(a, b):
                  return gf[:, a * C_:b * C_]

              def ktree():
                  op = mybir.AluOpType.add
                  GP.tensor_tensor(sl(20, 30), sl(0, 10), sl(10, 20), op)
                  GP.tensor_tensor(sl(20, 25), sl(20, 25), sl(25, 30), op)
                  GP.tensor_tensor(sl(20, 22), sl(20, 22), sl(22, 24), op)
                  GP.tensor_tensor(sl(20, 21), sl(20, 21), sl(21, 22), op)
                  GP.tensor_tensor(sl(20, 21), sl(20, 21), sl(24, 25), op)
                  return sl(20, 21)

              # max_k h = a + max_k b  -> fn half A columns via PE transpose
              # (Pool has no max ALU op, so the k-max is one DVE reduce)
              bkc = gdst[:, :, :].rearrange("p k c -> p c k")[:, :, 0:20]
              fnb = wpool.tile([P, 128], F32, tag="fnb")
              V.reduce_max(fnb[:, 0:Cout], bkc, axis=mybir.AxisListType.X)
              GP.tensor_add(fnb[:, 0:Cout], fnb[:, 0:Cout], aT[:, 0:Cout])
              ptr = psp.tile([128, P], F32, tag="ps")
              T.transpose(ptr[0:Cout, :], fnb[:, 0:Cout], cs["eye"][:, :])
              psum_to_sbuf(fn[0:Cout, blk * P:(blk + 1) * P], ptr[0:Cout, :])

              # stats from b alone:
              #  s = K a + S_b ; q = a (s + S_b) + S_bq
              bsum = ktree()
              sfix = wpool.tile([P, 128], F32, tag="sfix")
              qfix = wpool.tile([P, 128], F32, tag="qfix")
              GP.tensor_scalar_mul(sfix[:, 0:Cout], aT[:, 0:Cout], float(K))
              GP.tensor_add(sfix[:, 0:Cout], sfix[:, 0:Cout], bsum)
              GP.tensor_add(qfix[:, 0:Cout], sfix[:, 0:Cout], bsum)
              GP.tensor_mul(qfix[:, 0:Cout], qfix[:, 0:Cout], aT[:, 0:Cout])
              T.matmul(acc_s[:, 0:Cout], lhsT=cs["nh128"][:, :],
                       rhs=sfix[:, 0:Cout], start=(blk == 0),
                       stop=(blk == nloc - 1), skip_group_check=True)
              S.activation(sl(0, 20), sl(0, 20),
                           mybir.ActivationFunctionType.Square)
              bqsum = ktree()
              GP.tensor_add(qfix[:, 0:Cout], qfix[:, 0:Cout], bqsum)
              T.matmul(acc_q[:, 0:Cout], lhsT=cs["nh128"][:, :],
                       rhs=qfix[:, 0:Cout], start=(blk == 0),
                       stop=(blk == nloc - 1), skip_group_check=True)

          sts = {}
          for i in range(nloc + 3):
              if i < nloc:
                  sts[i] = stageA(i)
              if i >= 2 and i - 2 < nloc:
                  stageB1(i - 2, sts[i - 2])
              if i >= 3:
                  stageB2(i - 3, sts.pop(i - 3))

          # ---- layer exchange + finalize ----
          # acc rows are -0.5 * sums; fix scale and transpose (1, C) -> (C, 2)
          srow = spool.tile([1, 256], F32, tag="srow")
          psum_to_sbuf(srow[:, 0:Cout], acc_s[:, 0:Cout])
          psum_to_sbuf(srow[:, 128:128 + Cout], acc_q[:, 0:Cout])
          pcl = psp.tile([128, 2], F32, tag="ps")
          T.matmul(pcl[0:Cout, 0:1], lhsT=srow[:, 0:Cout],
                   rhs=cs["one11"][:, :])
          T.matmul(pcl[0:Cout, 1:2], lhsT=srow[:, 128:128 + Cout],
                   rhs=cs["one11"][:, :])
          sq2 = spool.tile([128, 2], F32, tag="sq2")
          V.tensor_scalar_mul(sq2[0:Cout, :], pcl[0:Cout, :], -2.0)

          # pack: local half features + stats columns -> DRAM, pair AllGather
          ci_d, co_d = dram[f"_ci{l}"], dram[f"_co{l}"]
          Y.dma_start(ci_d[:, 0:HN], fn[0:Cout, 0:HN])
          Y.dma_start(ci_d[:, HN:HN + 2], sq2[0:Cout, :])
          if "cc" in ablate:
              Y.dma_start(co_d[0:Cout, :], ci_d[:, :])
              Y.dma_start(co_d[Cout:2 * Cout, :], ci_d[:, :])
          else:
              GP.collective_compute(
                  "AllGather", mybir.AluOpType.bypass,
                  replica_groups=RG,
                  ins=[ci_d.ap().opt()],
                  outs=[co_d.ap().opt()],
              )
          gc = 16 * l
          GP.dma_gather(fn[:, 0:FW].unsqueeze(1), co_d[:, :],
                        gidx[:, gc:gc + 8], num_idxs=128, num_idxs_reg=Cout,
                        elem_size=FW, elem_step=FW, transpose=False,
                        single_packet=False, queue_num=0)
          GP.dma_gather(fn[:, FW:2 * FW].unsqueeze(1), co_d[:, :],
                        gidx[:, gc + 8:gc + 16], num_idxs=128, num_idxs_reg=Cout,
                        elem_size=FW, elem_step=FW, transpose=False,
                        single_packet=False, queue_num=1)

          # combined stats -> GroupNorm affine; apply + leaky on both halves
          sq = spool.tile([128, 2], F32, tag="sqc")
          V.tensor_add(sq[0:Cout, :], fn[0:Cout, HN:HN + 2],
                       fn[0:Cout, FW + HN:FW + HN + 2])
          cnt = float(N * K * (Cout // G))
          scale_t, shift_t = gn_affine(
              tc, nc, spool, psp,
              sq[0:Cout, 0:1], sq[0:Cout, 1:2],
              cs[f"GHs{l}"], cs[f"GHu{l}"], cs[f"GT{l}"],
              cs[f"absg{l}"], cs[f"g{l}"], cs[f"beta{l}"], Cout, cnt)
          for off in (0, FW):
              S.activation(fn[0:Cout, off:off + HN], fn[0:Cout, off:off + HN],
                           mybir.ActivationFunctionType.Identity,
                           bias=shift_t[:, :], scale=scale_t[:, :])
              V.scalar_tensor_tensor(fn[0:Cout, off:off + HN],
                                     fn[0:Cout, off:off + HN], 0.2,
                                     fn[0:Cout, off:off + HN],
                                     op0=mybir.AluOpType.mult,
                                     op1=mybir.AluOpType.max)

      # ================= aggregation conv + pooling =================
      chunks = [(f1, 64), (f2, 64), (f3, 128)]
      gmax = [None, None]
      gmean = [None, None]
      for h in range(2):
          agg = fpool.tile([P, N], F32, tag="agg")
          sta = spool.tile([P, 4], F32, tag=f"sta{h}")
          stq = spool.tile([P, 2], F32, tag=f"stq{h}")
          for rr in range(4):
              kt = pk.tile([P, Qf], F32, tag="kt")
              for j0 in range(0, Qf, 512):
                  j1 = min(j0 + 512, Qf)
                  for ci, (ft, csz) in enumerate(chunks):
                      T.matmul(kt[:, j0:j1],
                               lhsT=r32(cs[f"WaT{h}{ci}"][:, :]),
                               rhs=r32(fcols(ft, csz, rr * Qf + j0, j1 - j0)),
                               start=(ci == 0), stop=(ci == 2))
              psum_to_sbuf(agg[:, rr * Qf:(rr + 1) * Qf], kt[:, :],
                           accum=sta[:, rr:rr + 1])
          scrq = wpool.tile([P, N], F32, tag="keysb")
          S.activation(scrq[:, 0:Hf], agg[:, 0:Hf],
                       mybir.ActivationFunctionType.Square,
                       accum_out=stq[:, 0:1])
          S.activation(scrq[:, Hf:N], agg[:, Hf:N],
                       mybir.ActivationFunctionType.Square,
                       accum_out=stq[:, 1:2])
          ssum = spool.tile([P, 1], F32, tag="ssum")
          qsum = spool.tile([P, 1], F32, tag="ssum")
          V.reduce_sum(ssum[:, :], sta[:, :], axis=mybir.AxisListType.X)
          V.reduce_sum(qsum[:, :], stq[:, :], axis=mybir.AxisListType.X)
          cnt = float(N * 32)
          scale_t, shift_t = gn_affine(
              tc, nc, spool, psp, ssum[:, :], qsum[:, :],
              cs[f"GHa{h}"], cs[f"GHa{h}"], cs[f"GTa{h}"],
              cs[f"ga{h}"], None, cs[f"betaa{h}"], 128, cnt, signed_scale=True)
          S.activation(agg[:, :], agg[:, :],
                       mybir.ActivationFunctionType.Identity,
                       bias=shift_t[:, :], scale=scale_t[:, :])
          V.scalar_tensor_tensor(agg[:, :], agg[:, :], 0.2, agg[:, :],
                                 op0=mybir.AluOpType.mult,
                                 op1=mybir.AluOpType.max)
          gm = spool.tile([P, 1], F32, tag=f"gmax{h}")
          V.reduce_max(gm[:, :], agg[:, :], axis=mybir.AxisListType.X)
          gmax[h] = gm
          scr5 = wpool.tile([P, N], F32, tag="keysb")
          acc = spool.tile([P, 1], F32, tag=f"gmean{h}")
          S.activation(scr5[:, :], agg[:, :],
                       mybir.ActivationFunctionType.Identity,
                       accum_out=acc[:, :])
          V.tensor_scalar_mul(acc[:, :], acc[:, :], 1.0 / N)
          gmean[h] = acc

      # ================= head =================
      def gn_vec(z_sb, C, pref, cnt):
          zsq = spool.tile([C, 1], F32, tag="zsq")
          S.activation(zsq[:, :], z_sb, mybir.ActivationFunctionType.Square)
          scale_t, shift_t = gn_affine(
              tc, nc, spool, psp, z_sb, None,
              cs[f"GH{pref}"], cs[f"GH{pref}"], cs[f"GT{pref}"],
              cs[f"g{pref}"], None, cs[f"beta{pref}"], C, cnt,
              signed_scale=True, qsum_ap=zsq[:, :])
          out = spool.tile([C, 1], F32, tag="zv")
          S.activation(out[:, :], z_sb, mybir.ActivationFunctionType.Identity,
                       bias=shift_t[:, :], scale=scale_t[:, :])
          tmp = spool.tile([C, 1], F32, tag="zv2")
          V.tensor_scalar_mul(tmp[:, :], out[:, :], 0.2)
          V.tensor_max(out[:, :], out[:, :], tmp[:, :])
          return out

      def mm_vec(lhsT_list, rhs_list, M):
          pz = psp.tile([M, 1], F32, tag="ps")
          n = len(lhsT_list)
          for ci, (lt, rh) in enumerate(zip(lhsT_list, rhs_list)):
              T.matmul(pz[:, :], lhsT=lt, rhs=rh, start=(ci == 0),
                       stop=(ci == n - 1))
          z = spool.tile([M, 1], F32, tag="zv3")
          V.tensor_copy(z[:, :], pz[:, :])
          return z

      c1 = mm_vec([cs["Wc1T"][:, :]], [cond_sb[:, :]], 64)
      c1n = gn_vec(c1[:, :], 64, "c1", 8.0)
      c2 = mm_vec([cs["Wc2T"][:, :]], [c1n[:, :]], 64)
      c2n = gn_vec(c2[:, :], 64, "c2", 8.0)

      zvecs = [gmax[0], gmax[1], gmean[0], gmean[1], c2n]
      z1n = []
      for h in range(2):
          z1 = mm_vec([cs[f"Ws1T{h}{ci}"][:, :] for ci in range(5)],
                      [zv[:, :] for zv in zvecs], 128)
          z1n.append(gn_vec(z1[:, :], 128, f"z1{h}", 32.0))
      z2 = mm_vec([cs[f"Ws2T{h}"][:, :] for h in range(2)],
                  [z1n[h][:, :] for h in range(2)], 128)
      z2n = gn_vec(z2[:, :], 128, "z2", 16.0)
      z3 = mm_vec([cs["Ws3T"][:, :]], [z2n[:, :]], 64)
      z3n = gn_vec(z3[:, :], 64, "z3", 8.0)
      zo = mm_vec([cs["Ws4T"][:, :]], [z3n[:, :]], 2)
      V.tensor_add(zo[:, :], zo[:, :], cs["bs4"][:, :])
      Y.dma_start(out_d[:, :], zo[:, :])


def gn_affine(tc, nc, spool, psp, ssum_ap, qsum_ap_in, GHs, GHu, GT,
              absg, g_signed, beta, C, cnt, signed_scale=False, qsum_ap=None):
    """Compute per-channel scale/shift tiles for the GroupNorm affine.

    scale = gamma' * rsqrt(var_g + eps); shift = beta - gamma * mu_g * rsqrt.
    gamma' = |gamma| when signed_scale=False (max-commute trick), else gamma.
    """
    V = nc.vector
    S = nc.scalar
    T = nc.tensor
    F = mybir.ActivationFunctionType
    if qsum_ap is None:
        qsum_ap = qsum_ap_in
    pg = psp.tile([G, 2], F32, tag="ps")
    T.matmul(pg[:, 0:1], lhsT=GHs[0:C, :], rhs=ssum_ap)
    T.matmul(pg[:, 1:2], lhsT=GHu[0:C, :], rhs=qsum_ap)
    sg = spool.tile([G, 2], F32, tag="sg")
    V.tensor_copy(sg[:, :], pg[:, :])
    pc = psp.tile([C, 2], F32, tag="ps")
    T.matmul(pc[:, :], lhsT=GT[:, 0:C], rhs=sg[:, :])
    sc = spool.tile([C, 2], F32, tag="sc")
    V.tensor_copy(sc[:, :], pc[:, :])
    mu = spool.tile([C, 1], F32, tag="mu")
    msq = spool.tile([C, 1], F32, tag="msq")
    V.tensor_scalar_mul(mu[:, :], sc[:, 0:1], 1.0 / cnt)
    V.tensor_scalar_mul(msq[:, :], sc[:, 1:2], 1.0 / cnt)
    var = spool.tile([C, 1], F32, tag="var")
    V.tensor_mul(var[:, :], mu[:, :], mu[:, :])
    V.tensor_sub(var[:, :], msq[:, :], var[:, :])
    V.tensor_scalar_add(var[:, :], var[:, :], EPS)
    rec = spool.tile([C, 1], F32, tag="rec")
    V.reciprocal(rec[:, :], var[:, :])
    rstd = spool.tile([C, 1], F32, tag="rstd")
    S.activation(rstd[:, :], rec[:, :], F.Sqrt)
    scale_t = spool.tile([C, 1], F32, tag="scale")
    gm = absg if not signed_scale else absg  # absg arg carries gamma' already
    V.tensor_mul(scale_t[:, :], rstd[:, :], gm[0:C, :])
    shift_t = spool.tile([C, 1], F32, tag="shift")
    V.tensor_mul(shift_t[:, :], mu[:, :], rstd[:, :])
    gsig = g_signed if g_signed is not None else absg
    V.tensor_mul(shift_t[:, :], shift_t[:, :], gsig[0:C, :])
    V.tensor_sub(shift_t[:, :], beta[0:C, :], shift_t[:, :])
    return scale_t, shift_t


# ----------------------------------------------------------------------------
# host entry point
# ----------------------------------------------------------------------------
_BUILT = {}


def make_in_maps(inputs, consts, B, n_cores):
    x = np.asarray(inputs["x"], np.float32)
    cond = np.asarray(inputs["cond"], np.float32)
    in_maps = []
    for c in range(n_cores):
        b, rank = c // 2, c % 2
        m = dict(consts)
        m["x"] = _sf(np.roll(x[b], -rank * HN, axis=1))
        m["cond"] = _sf(cond[b][:, None])
        m["gidx"] = host_gidx(rank)
        in_maps.append(m)
    return in_maps


def kernel(**inputs):
    from concourse.bass_utils import run_bass_kernel_spmd

    x = np.asarray(inputs["x"], np.float32)
    B, Cin, N = x.shape
    n_cores = 2 * B
    key = (N, B)
    if key not in _BUILT:
        _BUILT[key] = build_nc(N, num_devices=n_cores)
    nc = _BUILT[key]

    consts = host_prep(inputs, N)
    in_maps = make_in_maps(inputs, consts, B, n_cores)
    res = run_bass_kernel_spmd(nc, in_maps, core_ids=list(range(n_cores)))
    out = np.stack([res.results[2 * b]["out"][:, 0] for b in range(B)], axis=0)
    return out.astype(np.float32)


if __name__ == "__main__":
    import reference
    inputs = reference.setup_inputs()
    inputs = {k: np.asarray(v) for k, v in inputs.items()}
    got = kernel(**inputs)
    exp = np.asarray(reference.reference(**{k: np.asarray(v) for k, v in inputs.items()}))
    err = np.abs(got - exp).max() / (np.abs(exp).max() + 1e-9)
    print("out:", got)
    print("exp:", exp)
    print("Relative error:", err)
